# revision 21
# baseline (speedup 1.0000x reference)
"""Trainium2 Bass kernel for nn_Attention_45930380263558.

Attention module (EfficientViT-style attention with a gathered relative
position bias) over x:[16, 1024, 512]:
    qkv = x @ qkv_w + qkv_b                  # [B, N, 2048]
    split per head h: q,k (64), v (128)
    attn = softmax(q k^T * 64^-0.5 + bias_h[gather])
    out  = (attn @ v) per head, concat -> @ proj_w + proj_b

Sharding: data-parallel over batch, 2 batches per core on 8 NeuronCores.
No collectives. Each core computes its 2 batches fully.

Performance structure (v7):
  - fp16 operands on TensorE, fp32 PSUM accumulation.
  - The gathered bias table ([H, N, N], 16 MB) is never streamed from
    HBM: bias[k, q] = E_h[|k0-q0|*32 + |k1-q1|] is block-Toeplitz, so
    every row of the [N, N] table is a slice of a per-head replicated-
    shifted strip strip4[a*32+k1, w] = strip[k1, w - a*32] ([128,2112]).
    The bias multiply reads the strip slice directly - zero bias DMAs.
  - Head-PAIR attention with q-half panels: heads 2m / 2m+1 live at
    partitions 0-63 / 64-127 of the same qk m-tile, so their K=64 S
    matmuls land in different PE row groups and execute CONCURRENTLY
    (row tiling; the second matmul adds ~4ns). Both write one 2-bank
    [128,1024] PSUM tile (h0 -> cols 0:512, h1 -> 512:1024) which a
    single ScalarE exp consumes. The M=1 rowsum matmuls are col-tiled
    at PSUM partitions 0/64 (PE col groups 0/2) and also run as a
    concurrent pair. PV (K=128, full PE) lags 2 iterations so the
    in-order PE never waits on ScalarE/VectorE.
  - PSUM (8 banks): s 2x[128,1024]=4, o 2x[128,512]=2, rs 2x[65,512]=2.
  - Cold-start: only a minimal qkv prefix (first head-pair's qk tiles +
    all v tiles of batch 0) runs before attention starts; every other
    qkv/projection tile is dribbled into the attention-phase PE stream
    as fillers (batch 1 qkv during batch 0's attention, batch 0
    projection during batch 1's).
  - Reciprocal: no HW divide, and the custom-DVE fast-reciprocal
    miscompiles, so rowsums take a DRAM round-trip into a [128,8]
    layout where a magic-seed + 2-step Newton iteration runs partition-
    parallel on VectorE, then broadcast back via DMA.
  - Softmax max-subtraction skipped (logits bounded ~|7|).
"""

import os
import sys

for _p in ("/opt/trn_rl_repo",):
    if _p not in sys.path and os.path.isdir(_p):
        sys.path.insert(0, _p)

from contextlib import ExitStack

import numpy as np

import concourse.bass as bass
import concourse.tile as tile
from concourse import bacc, mybir
from concourse.bass_utils import run_bass_kernel_spmd

F32 = mybir.dt.float32
F16 = mybir.dt.float16
BF16 = mybir.dt.bfloat16
I32 = mybir.dt.int32

N_CORES = 8
B = 16
B_LOC = B // N_CORES  # 2
N = 1024  # tokens
D = 512  # model dim
H = 8  # heads
DK = 64  # key dim
DV = 128  # value dim per head
SCALE = DK ** -0.5
NT = N // 128  # 8 token tiles
DC = D // 128  # 4 dim chunks
QH = 2  # q halves of 512
RES = 32  # grid side; N = RES*RES
STRIP_W = (2 * RES - 1) * RES  # 2016
STRIP4_W = STRIP_W + 96  # 2112

# module-level stash so test.py can read timing info
LAST_RESULT = None


def _ensure_axon_hooks_module():
    """bass_utils' trace path imports antenv.axon_hooks, which some agent
    images lack. Provide a minimal get/set pair so trace degrades
    gracefully (hook=None -> tracing skipped) instead of crashing."""
    try:
        import antenv.axon_hooks  # noqa: F401
        return
    except ImportError:
        pass
    import types

    import antenv

    m = types.ModuleType("antenv.axon_hooks")
    m._hook = None

    def set_axon_ntff_profile_hook(h):
        m._hook = h

    def get_axon_ntff_profile_hook():
        return m._hook

    m.set_axon_ntff_profile_hook = set_axon_ntff_profile_hook
    m.get_axon_ntff_profile_hook = get_axon_ntff_profile_hook
    sys.modules["antenv.axon_hooks"] = m
    antenv.axon_hooks = m


_ensure_axon_hooks_module()


def build_program(use_qkv_bias: bool, use_proj_bias: bool):
    nc = bacc.Bacc("TRN2", target_bir_lowering=False, debug=False,
                   num_devices=N_CORES)

    xT_d = nc.dram_tensor("xT", [B_LOC, DC, 128, N], F16, kind="ExternalInput").ap()
    w_qk_d = nc.dram_tensor("w_qk", [DC, 128, N], F16, kind="ExternalInput").ap()
    w_v_d = nc.dram_tensor("w_v", [DC, 128, N], F16, kind="ExternalInput").ap()
    strip_d = nc.dram_tensor("strip", [H, 128, STRIP4_W], BF16, kind="ExternalInput").ap()
    w_proj_d = nc.dram_tensor("w_proj", [H, 128, D], BF16, kind="ExternalInput").ap()
    ones_d = nc.dram_tensor("ones", [128, N], BF16, kind="ExternalInput").ap()
    rs_scr = nc.dram_tensor("rs_scratch", [B_LOC, H // 2, QH, N], F32).ap()
    inv_scr = nc.dram_tensor("inv_scratch", [B_LOC, H // 2, QH, N], F32).ap()
    out_d = nc.dram_tensor("out", [B_LOC, N, D], F32, kind="ExternalOutput").ap()
    if use_qkv_bias:
        qk_bias_d = nc.dram_tensor("qk_bias", [1, N], BF16, kind="ExternalInput").ap()
        v_bias_d = nc.dram_tensor("v_bias", [1, N], BF16, kind="ExternalInput").ap()
    if use_proj_bias:
        proj_bias_d = nc.dram_tensor("proj_bias", [1, D], BF16, kind="ExternalInput").ap()

    with tile.TileContext(nc) as tc, ExitStack() as ctx:
        consts = ctx.enter_context(tc.tile_pool(name="consts", bufs=1))
        xp = ctx.enter_context(tc.tile_pool(name="xp", bufs=2))
        qkp = ctx.enter_context(tc.tile_pool(name="qkp", bufs=2))
        vp = ctx.enter_context(tc.tile_pool(name="vp", bufs=2))
        onp = ctx.enter_context(tc.tile_pool(name="onp", bufs=2))
        stripp = ctx.enter_context(tc.tile_pool(name="stripp", bufs=2))
        ep = ctx.enter_context(tc.tile_pool(name="ep", bufs=5))
        ptp = ctx.enter_context(tc.tile_pool(name="ptp", bufs=6))
        osbp = ctx.enter_context(tc.tile_pool(name="osbp", bufs=2))
        invp = ctx.enter_context(tc.tile_pool(name="invp", bufs=2))
        bcp = ctx.enter_context(tc.tile_pool(name="bcp", bufs=2))
        outp = ctx.enter_context(tc.tile_pool(name="outp", bufs=3))

        ps_s = ctx.enter_context(tc.tile_pool(name="ps_s", bufs=2, space="PSUM"))
        ps_o = ctx.enter_context(tc.tile_pool(name="ps_o", bufs=1, space="PSUM"))
        ps_rs = ctx.enter_context(tc.tile_pool(name="ps_rs", bufs=2, space="PSUM"))

        # ---- constants ----
        w_qk_t = consts.tile([128, DC, N], F16)
        w_v_t = consts.tile([128, DC, N], F16)
        for kc in range(DC):
            nc.sync.dma_start(out=w_qk_t[:, kc, :], in_=w_qk_d[kc])
            nc.sync.dma_start(out=w_v_t[:, kc, :], in_=w_v_d[kc])
        w_proj_t = consts.tile([128, H, D], BF16)
        ones_t = consts.tile([128, N], BF16)
        nc.sync.dma_start(out=ones_t, in_=ones_d)
        ones_col = ones_t[:, 0:1]
        ones_row = ones_t[0:1, 0:128]
        if use_qkv_bias:
            qk_bias_t = consts.tile([1, N], BF16)
            nc.sync.dma_start(out=qk_bias_t, in_=qk_bias_d)
            v_bias_t = consts.tile([1, N], BF16)
            nc.sync.dma_start(out=v_bias_t, in_=v_bias_d)
            ones_n = ones_t[0:1, :]
        if use_proj_bias:
            proj_bias_t = consts.tile([1, D], BF16)
            nc.sync.dma_start(out=proj_bias_t, in_=proj_bias_d)

        x_ts = [None] * B_LOC
        qk_sbs = [None] * B_LOC
        v_sbs = [None] * B_LOC
        on8s = [None] * B_LOC

        def emit_qk_tile(b, mt):
            st = ps_s.tile([128, N], F32, tag="s")
            for nt in range(QH):
                for kc in range(DC):
                    nc.tensor.matmul(
                        st[:, nt * 512:(nt + 1) * 512],
                        lhsT=w_qk_t[:, kc, mt * 128:(mt + 1) * 128],
                        rhs=x_ts[b][:, kc, nt * 512:(nt + 1) * 512],
                        start=(kc == 0),
                        stop=(kc == DC - 1 and not use_qkv_bias),
                    )
                if use_qkv_bias:
                    nc.tensor.matmul(
                        st[:, nt * 512:(nt + 1) * 512],
                        lhsT=qk_bias_t[:, mt * 128:(mt + 1) * 128],
                        rhs=ones_n[:, nt * 512:(nt + 1) * 512],
                        start=False, stop=True,
                    )
            with nc.allow_low_precision(reason="fp16 activations"):
                nc.vector.tensor_copy(qk_sbs[b][:, mt, :], st)

        def emit_v_tile(b, tt):
            st = ps_s.tile([128, N], F32, tag="s")
            for nt in range(QH):
                for kc in range(DC):
                    nc.tensor.matmul(
                        st[:, nt * 512:(nt + 1) * 512],
                        lhsT=x_ts[b][:, kc, tt * 128:(tt + 1) * 128],
                        rhs=w_v_t[:, kc, nt * 512:(nt + 1) * 512],
                        start=(kc == 0),
                        stop=(kc == DC - 1 and not use_qkv_bias),
                    )
                if use_qkv_bias:
                    nc.tensor.matmul(
                        st[:, nt * 512:(nt + 1) * 512],
                        lhsT=ones_n[:, tt * 128:(tt + 1) * 128],
                        rhs=v_bias_t[:, nt * 512:(nt + 1) * 512],
                        start=False, stop=True,
                    )
            with nc.allow_low_precision(reason="fp16 activations"):
                nc.vector.tensor_copy(v_sbs[b][:, tt, :], st)

        def emit_proj_qt(b, qt):
            st = ps_s.tile([128, N], F32, tag="s")
            for h2 in range(H):
                nc.tensor.matmul(
                    st[:, 0:512],
                    lhsT=on8s[b][:, h2, qt * 128:(qt + 1) * 128],
                    rhs=w_proj_t[:, h2, :],
                    start=(h2 == 0),
                    stop=(h2 == H - 1 and not use_proj_bias),
                )
            if use_proj_bias:
                nc.tensor.matmul(
                    st[:, 0:512],
                    lhsT=ones_row,
                    rhs=proj_bias_t,
                    start=False, stop=True,
                )
            ot = outp.tile([128, 512], F32)
            nc.vector.tensor_copy(ot, st[:, 0:512])
            nc.sync.dma_start(
                out=out_d[b, qt * 128:(qt + 1) * 128, :], in_=ot)

        # ---- load x; minimal qkv prefix for batch 0 ----
        for b in range(B_LOC):
            x_ts[b] = xp.tile([128, DC, N], F16, name="x_t")
            qk_sbs[b] = qkp.tile([128, NT, N], F16, name="qk_sb")
            v_sbs[b] = vp.tile([128, NT, N], BF16, name="v_sb")
            on8s[b] = onp.tile([128, H, N], BF16, name="on8")
        for kc in range(DC):
            nc.sync.dma_start(out=x_ts[0][:, kc, :], in_=xT_d[0, kc])
        emit_qk_tile(0, 0)
        emit_qk_tile(0, 4)
        for kc in range(DC):  # b1's x: off the startup critical path
            nc.sync.dma_start(out=x_ts[1][:, kc, :], in_=xT_d[1, kc])
        nc.sync.dma_start(out=w_proj_t, in_=w_proj_d.transpose([1, 0, 2]))
        for tt in range(NT):
            emit_v_tile(0, tt)

        # ---- attention: head pairs x q-half panels, batch-outer ----
        def emit_epilogue(o0, o1, rs_ps, hp, qh, b):
            h0, h1 = 2 * hp, 2 * hp + 1
            # free the o PSUM banks first
            o_sb = osbp.tile([128, N], F32)
            nc.vector.tensor_copy(o_sb[:, 0:512], o0)
            nc.vector.tensor_copy(o_sb[:, 512:N], o1)
            inv_t = invp.tile([128, 512], F32)
            nc.vector.tensor_copy(inv_t[0:1, :], rs_ps[0:1, :])
            nc.vector.tensor_copy(inv_t[64:65, :], rs_ps[64:65, :])
            nc.sync.dma_start(out=rs_scr[b, hp, qh, 0:512], in_=inv_t[0:1, :])
            nc.sync.dma_start(out=rs_scr[b, hp, qh, 512:N], in_=inv_t[64:65, :])
            # reload as [128, 8]: Newton reciprocal runs partition-parallel
            rsw = invp.tile([128, 8], F32, name="rsw")
            base = ((b * (H // 2) + hp) * QH + qh) * N
            nc.sync.dma_start(out=rsw, in_=bass.AP(
                tensor=rs_scr.tensor, offset=rs_scr.offset + base,
                ap=[[8, 128], [1, 8]]))
            yw = invp.tile([128, 8], F32, name="yw")
            tw = invp.tile([128, 8], F32, name="tw")
            nc.gpsimd.tensor_scalar(
                yw.bitcast(I32), rsw.bitcast(I32),
                scalar1=-1, scalar2=0x7EF311C3,
                op0=mybir.AluOpType.mult, op1=mybir.AluOpType.add)
            for _ in range(2):  # y *= (2 - x*y)
                nc.gpsimd.tensor_tensor(tw, rsw, yw, op=mybir.AluOpType.mult)
                nc.gpsimd.tensor_scalar(
                    tw, tw, scalar1=-1.0, scalar2=2.0,
                    op0=mybir.AluOpType.mult, op1=mybir.AluOpType.add)
                nc.gpsimd.tensor_tensor(yw, yw, tw, op=mybir.AluOpType.mult)
            nc.sync.dma_start(out=inv_scr[b, hp, qh], in_=yw)
            bc_t = bcp.tile([128, N], F32)
            nc.sync.dma_start(out=bc_t, in_=bass.AP(
                tensor=inv_scr.tensor, offset=inv_scr.offset + base,
                ap=[[0, 128], [1, N]]))
            with nc.allow_low_precision(reason="bf16 normalized O^T"):
                nc.gpsimd.tensor_tensor(
                    on8s[b][:, h0, qh * 512:(qh + 1) * 512],
                    o_sb[:, 0:512], bc_t[:, 0:512], op=mybir.AluOpType.mult)
                nc.gpsimd.tensor_tensor(
                    on8s[b][:, h1, qh * 512:(qh + 1) * 512],
                    o_sb[:, 512:N], bc_t[:, 512:N], op=mybir.AluOpType.mult)

        pv_queue = []  # (pt0, pt1, kc, v0, v1, o0, o1, rs_ps, epi)

        def pop_pv():
            pt0, pt1, kcp, v0, v1, o0, o1, rs_ps, epi = pv_queue.pop(0)
            nc.tensor.matmul(o0, lhsT=v0, rhs=pt0,
                             start=(kcp == 0), stop=(kcp == NT - 1))
            nc.tensor.matmul(o1, lhsT=v1, rhs=pt1,
                             start=(kcp == 0), stop=(kcp == NT - 1))
            # rowsum pair col-tiled at PSUM partitions 0/64 -> concurrent
            nc.tensor.matmul(rs_ps[0:1, :], lhsT=ones_col, rhs=pt0,
                             start=(kcp == 0), stop=(kcp == NT - 1),
                             skip_group_check=True)
            nc.tensor.matmul(rs_ps[64:65, :], lhsT=ones_col, rhs=pt1,
                             start=(kcp == 0), stop=(kcp == NT - 1),
                             skip_group_check=True)
            if epi is not None:
                emit_epilogue(*epi)

        for b in range(B_LOC):
            if b == 0:
                fillers = []
                for m2 in range(1, 4):  # head pair m2 needs qk m2 & 4+m2
                    fillers += [(emit_qk_tile, (0, m2)),
                                (emit_qk_tile, (0, 4 + m2))]
                fillers += [(emit_qk_tile, (1, mt)) for mt in range(NT)]
                fillers += [(emit_v_tile, (1, tt)) for tt in range(NT)]
            else:
                fillers = [(emit_proj_qt, (0, qt)) for qt in range(NT)]
            def load_strip(hp2):
                t = stripp.tile([128, 2, STRIP4_W], BF16, name="strip01")
                nc.sync.dma_start(out=t[:, 0, :], in_=strip_d[2 * hp2])
                nc.sync.dma_start(out=t[:, 1, :], in_=strip_d[2 * hp2 + 1])
                return t
            strip_next = load_strip(0)
            for hp in range(H // 2):
                h0, h1 = 2 * hp, 2 * hp + 1
                strip01 = strip_next
                if hp + 1 < H // 2:
                    strip_next = load_strip(hp + 1)
                qT0 = qk_sbs[b][0:64, hp, :]
                kT0 = qk_sbs[b][0:64, 4 + hp, :]
                qT1 = qk_sbs[b][64:128, hp, :]
                kT1 = qk_sbs[b][64:128, 4 + hp, :]
                for qh in range(QH):
                    o0 = ps_o.tile([128, 512], F32, tag="o0")
                    o1 = ps_o.tile([128, 512], F32, tag="o1")
                    rs_ps = ps_rs.tile([65, 512], F32, tag="rs")
                    for kc in range(NT):
                        base_c = (RES - 1 - 4 * kc) * RES + qh * 512
                        st = ps_s.tile([128, N], F32, tag="s")
                        # the two heads' K=64 S matmuls: PE row groups
                        # 0-63 / 64-127 -> execute concurrently
                        nc.tensor.matmul(
                            st[:, 0:512],
                            lhsT=kT0[:, kc * 128:(kc + 1) * 128],
                            rhs=qT0[:, qh * 512:(qh + 1) * 512],
                            start=True, stop=True,
                        )
                        nc.tensor.matmul(
                            st[:, 512:N],
                            lhsT=kT1[:, kc * 128:(kc + 1) * 128],
                            rhs=qT1[:, qh * 512:(qh + 1) * 512],
                            start=True, stop=True,
                        )
                        e_t = ep.tile([128, N], BF16)
                        with nc.allow_low_precision(reason="bf16 exp"):
                            nc.scalar.activation(
                                e_t, st, mybir.ActivationFunctionType.Exp,
                                scale=SCALE)
                        pt_t = ptp.tile([128, N], BF16, name="pt_t")
                        with nc.allow_low_precision(reason="bf16 P^T"):
                            nc.vector.tensor_tensor(
                                pt_t.rearrange("p (a c) -> p a c", a=2),
                                e_t.rearrange("p (a c) -> p a c", a=2),
                                strip01[:, :, base_c:base_c + 512],
                                op=mybir.AluOpType.mult)
                        pt0 = pt_t[:, 0:512]
                        pt1 = pt_t[:, 512:N]
                        epi = ((o0, o1, rs_ps, hp, qh, b)
                               if kc == NT - 1 else None)
                        pv_queue.append(
                            (pt0, pt1, kc,
                             v_sbs[b][:, kc, h0 * 128:(h0 + 1) * 128],
                             v_sbs[b][:, kc, h1 * 128:(h1 + 1) * 128],
                             o0, o1, rs_ps, epi))
                        if len(pv_queue) > 3:
                            pop_pv()
                        if kc in ((2, 5, 7) if b == 0 else (7,)):
                            if fillers:
                                f, args = fillers.pop(0)
                                f(*args)
            for f, args in fillers:
                f(*args)
        while pv_queue:
            pop_pv()

        # ---- projection for batch 1 (batch 0 ran as fillers) ----
        for qt in range(NT):
            emit_proj_qt(1, qt)

    nc.compile()
    return nc


def _prep_core_inputs(x, qkv_w, qkv_b, proj_w, proj_b, attn_biases, bias_idxs):
    """Host-side layout preparation. Returns (shared, per_core_xT, flags)."""
    x = np.ascontiguousarray(np.asarray(x, np.float32))
    qkv_w = np.asarray(qkv_w, np.float32)
    qkv_b = np.asarray(qkv_b, np.float32)
    proj_w = np.asarray(proj_w, np.float32)
    proj_b = np.asarray(proj_b, np.float32)
    attn_biases = np.asarray(attn_biases, np.float32)
    import ml_dtypes
    bf16 = ml_dtypes.bfloat16

    # qkv_w columns: per head 256 = [q 64 | k 64 | v 128]
    Wh = qkv_w.reshape(D, H, 256)
    w_q = Wh[:, :, :DK].reshape(D, H * DK)            # q chans h-major
    w_k = Wh[:, :, DK:2 * DK].reshape(D, H * DK)
    w_qk = np.concatenate([w_q, w_k], axis=1)          # [512, 1024]
    w_v = Wh[:, :, 2 * DK:].reshape(D, H * DV)         # [512, 1024]

    bh = qkv_b.reshape(H, 256)
    qk_bias = np.concatenate([bh[:, :DK].reshape(-1), bh[:, DK:2 * DK].reshape(-1)])
    v_bias = bh[:, 2 * DK:].reshape(-1)

    # strip_h[k1, u*32 + q1] = exp(attn_biases[h])[|u-31|*32 + |k1-q1|]
    E = np.exp(attn_biases)                            # [H, 1024]
    u = np.arange(2 * RES - 1)
    d0 = np.abs(u - (RES - 1))                         # [63]
    r = np.arange(RES)
    rel1 = np.abs(r[:, None] - r[None, :])             # [32, 32] (k1, q1)
    idx = d0[None, :, None] * RES + rel1[:, None, :]   # [32, 63, 32]
    strip0 = E[:, idx.reshape(RES, STRIP_W)]           # [H, 32, 2016]
    # replicated-shifted strip: strip4[h, a*32+k1, w] = strip0[h, k1, w-a*32]
    strip = np.zeros((H, 128, STRIP4_W), np.float32)
    for a in range(4):
        strip[:, a * RES:(a + 1) * RES, a * RES:a * RES + STRIP_W] = strip0
    strip = strip.astype(bf16)

    shared = {
        "ones": np.ones((128, N), bf16),
        "w_qk": np.ascontiguousarray(w_qk.reshape(DC, 128, H * DK * 2)).astype(np.float16),
        "w_v": np.ascontiguousarray(w_v.reshape(DC, 128, H * DV)).astype(np.float16),
        "strip": strip,
        "w_proj": np.ascontiguousarray(proj_w.reshape(H, 128, D)).astype(bf16),
    }
    use_qkv_bias = bool(np.any(qkv_b))
    use_proj_bias = bool(np.any(proj_b))
    if use_qkv_bias:
        shared["qk_bias"] = qk_bias.reshape(1, N).astype(bf16)
        shared["v_bias"] = v_bias.reshape(1, N).astype(bf16)
    if use_proj_bias:
        shared["proj_bias"] = proj_b.reshape(1, D).astype(bf16)

    # x^T per core: [B_LOC, DC, 128, N] fp16
    xT = np.ascontiguousarray(x.transpose(0, 2, 1)).reshape(B, DC, 128, N)
    xT = xT.astype(np.float16)
    per_core = [xT[c * B_LOC:(c + 1) * B_LOC] for c in range(N_CORES)]
    return shared, per_core, use_qkv_bias, use_proj_bias


def kernel(x, qkv_w, qkv_b, proj_w, proj_b, attn_biases, bias_idxs):
    global LAST_RESULT
    shared, per_core, use_qkv_bias, use_proj_bias = _prep_core_inputs(
        x, qkv_w, qkv_b, proj_w, proj_b, attn_biases, bias_idxs)

    nc = build_program(use_qkv_bias, use_proj_bias)

    in_maps = [dict(shared, xT=per_core[c]) for c in range(N_CORES)]
    trace = bool(os.environ.get("BASS_TRACE"))
    res = run_bass_kernel_spmd(nc, in_maps, core_ids=list(range(N_CORES)),
                               trace=trace)
    LAST_RESULT = res
    out = np.concatenate([res.results[c]["out"] for c in range(N_CORES)], axis=0)
    return np.ascontiguousarray(out.astype(np.float32))


# revision 22
# speedup vs baseline: 1.0870x; 1.0870x over previous
"""Trainium2 Bass kernel for nn_Attention_45930380263558.

Attention module (EfficientViT-style attention with a gathered relative
position bias) over x:[16, 1024, 512]:
    qkv = x @ qkv_w + qkv_b                  # [B, N, 2048]
    split per head h: q,k (64), v (128)
    attn = softmax(q k^T * 64^-0.5 + bias_h[gather])
    out  = (attn @ v) per head, concat -> @ proj_w + proj_b

Sharding: data-parallel over batch, 2 batches per core on 8 NeuronCores.
No collectives. Each core computes its 2 batches fully.

Performance structure (v7):
  - fp16 operands on TensorE, fp32 PSUM accumulation.
  - The gathered bias table ([H, N, N], 16 MB) is never streamed from
    HBM: bias[k, q] = E_h[|k0-q0|*32 + |k1-q1|] is block-Toeplitz, so
    every row of the [N, N] table is a slice of a per-head replicated-
    shifted strip strip4[a*32+k1, w] = strip[k1, w - a*32] ([128,2112]).
    The bias multiply reads the strip slice directly - zero bias DMAs.
  - Head-PAIR attention with q-half panels: heads 2m / 2m+1 live at
    partitions 0-63 / 64-127 of the same qk m-tile, so their K=64 S
    matmuls land in different PE row groups and execute CONCURRENTLY
    (row tiling; the second matmul adds ~4ns). Both write one 2-bank
    [128,1024] PSUM tile (h0 -> cols 0:512, h1 -> 512:1024) which a
    single ScalarE exp consumes. The M=1 rowsum matmuls are col-tiled
    at PSUM partitions 0/64 (PE col groups 0/2) and also run as a
    concurrent pair. PV (K=128, full PE) lags 2 iterations so the
    in-order PE never waits on ScalarE/VectorE.
  - PSUM (8 banks): s 2x[128,1024]=4, o 2x[128,512]=2, rs 2x[65,512]=2.
  - Cold-start: only a minimal qkv prefix (first head-pair's qk tiles +
    all v tiles of batch 0) runs before attention starts; every other
    qkv/projection tile is dribbled into the attention-phase PE stream
    as fillers (batch 1 qkv during batch 0's attention, batch 0
    projection during batch 1's).
  - Reciprocal: no HW divide, and the custom-DVE fast-reciprocal
    miscompiles, so rowsums take a DRAM round-trip into a [128,8]
    layout where a magic-seed + 2-step Newton iteration runs partition-
    parallel on VectorE, then broadcast back via DMA.
  - Softmax max-subtraction skipped (logits bounded ~|7|).
"""

import os
import sys

for _p in ("/opt/trn_rl_repo",):
    if _p not in sys.path and os.path.isdir(_p):
        sys.path.insert(0, _p)

from contextlib import ExitStack

import numpy as np

import concourse.bass as bass
import concourse.tile as tile
from concourse import bacc, mybir
from concourse.bass_utils import run_bass_kernel_spmd

F32 = mybir.dt.float32
F16 = mybir.dt.float16
BF16 = mybir.dt.bfloat16
I32 = mybir.dt.int32

N_CORES = 8
B = 16
B_LOC = B // N_CORES  # 2
N = 1024  # tokens
D = 512  # model dim
H = 8  # heads
DK = 64  # key dim
DV = 128  # value dim per head
SCALE = DK ** -0.5
NT = N // 128  # 8 token tiles
DC = D // 128  # 4 dim chunks
QH = 2  # q halves of 512
RES = 32  # grid side; N = RES*RES
STRIP_W = (2 * RES - 1) * RES  # 2016
STRIP4_W = STRIP_W + 96  # 2112

# module-level stash so test.py can read timing info
LAST_RESULT = None


def _ensure_axon_hooks_module():
    """bass_utils' trace path imports antenv.axon_hooks, which some agent
    images lack. Provide a minimal get/set pair so trace degrades
    gracefully (hook=None -> tracing skipped) instead of crashing."""
    try:
        import antenv.axon_hooks  # noqa: F401
        return
    except ImportError:
        pass
    import types

    import antenv

    m = types.ModuleType("antenv.axon_hooks")
    m._hook = None

    def set_axon_ntff_profile_hook(h):
        m._hook = h

    def get_axon_ntff_profile_hook():
        return m._hook

    m.set_axon_ntff_profile_hook = set_axon_ntff_profile_hook
    m.get_axon_ntff_profile_hook = get_axon_ntff_profile_hook
    sys.modules["antenv.axon_hooks"] = m
    antenv.axon_hooks = m


_ensure_axon_hooks_module()


def build_program(use_qkv_bias: bool, use_proj_bias: bool):
    nc = bacc.Bacc("TRN2", target_bir_lowering=False, debug=False,
                   num_devices=N_CORES)

    xT_d = nc.dram_tensor("xT", [B_LOC, DC, 128, N], F16, kind="ExternalInput").ap()
    w_qk_d = nc.dram_tensor("w_qk", [DC, 128, N], F16, kind="ExternalInput").ap()
    w_v_d = nc.dram_tensor("w_v", [DC, 128, N], F16, kind="ExternalInput").ap()
    strip_d = nc.dram_tensor("strip", [H, 128, STRIP4_W], BF16, kind="ExternalInput").ap()
    w_proj_d = nc.dram_tensor("w_proj", [H, 128, D], BF16, kind="ExternalInput").ap()
    ones_d = nc.dram_tensor("ones", [128, N], BF16, kind="ExternalInput").ap()
    rs_scr = nc.dram_tensor("rs_scratch", [B_LOC, H // 2, QH, N], F32).ap()
    inv_scr = nc.dram_tensor("inv_scratch", [B_LOC, H // 2, QH, N], F32).ap()
    out_d = nc.dram_tensor("out", [B_LOC, N, D], F32, kind="ExternalOutput").ap()
    if use_qkv_bias:
        qk_bias_d = nc.dram_tensor("qk_bias", [1, N], BF16, kind="ExternalInput").ap()
        v_bias_d = nc.dram_tensor("v_bias", [1, N], BF16, kind="ExternalInput").ap()
    if use_proj_bias:
        proj_bias_d = nc.dram_tensor("proj_bias", [1, D], BF16, kind="ExternalInput").ap()

    with tile.TileContext(nc) as tc, ExitStack() as ctx:
        consts = ctx.enter_context(tc.tile_pool(name="consts", bufs=1))
        xp = ctx.enter_context(tc.tile_pool(name="xp", bufs=2))
        qkp = ctx.enter_context(tc.tile_pool(name="qkp", bufs=2))
        vp = ctx.enter_context(tc.tile_pool(name="vp", bufs=2))
        onp = ctx.enter_context(tc.tile_pool(name="onp", bufs=2))
        stripp = ctx.enter_context(tc.tile_pool(name="stripp", bufs=2))
        ep = ctx.enter_context(tc.tile_pool(name="ep", bufs=5))
        ptp = ctx.enter_context(tc.tile_pool(name="ptp", bufs=6))
        osbp = ctx.enter_context(tc.tile_pool(name="osbp", bufs=2))
        invp = ctx.enter_context(tc.tile_pool(name="invp", bufs=2))
        bcp = ctx.enter_context(tc.tile_pool(name="bcp", bufs=2))
        outp = ctx.enter_context(tc.tile_pool(name="outp", bufs=3))

        ps_s = ctx.enter_context(tc.tile_pool(name="ps_s", bufs=2, space="PSUM"))
        ps_o = ctx.enter_context(tc.tile_pool(name="ps_o", bufs=1, space="PSUM"))
        ps_rs = ctx.enter_context(tc.tile_pool(name="ps_rs", bufs=2, space="PSUM"))

        # ---- constants ----
        w_qk_t = consts.tile([128, DC, N], F16)
        w_v_t = consts.tile([128, DC, N], F16)
        for kc in range(DC):
            nc.sync.dma_start(out=w_qk_t[:, kc, :], in_=w_qk_d[kc])
            nc.sync.dma_start(out=w_v_t[:, kc, :], in_=w_v_d[kc])
        w_proj_t = consts.tile([128, H, D], BF16)
        ones_t = consts.tile([128, N], BF16)
        nc.sync.dma_start(out=ones_t, in_=ones_d)
        ones_col = ones_t[:, 0:1]
        ones_row = ones_t[0:1, 0:128]
        if use_qkv_bias:
            qk_bias_t = consts.tile([1, N], BF16)
            nc.sync.dma_start(out=qk_bias_t, in_=qk_bias_d)
            v_bias_t = consts.tile([1, N], BF16)
            nc.sync.dma_start(out=v_bias_t, in_=v_bias_d)
            ones_n = ones_t[0:1, :]
        if use_proj_bias:
            proj_bias_t = consts.tile([1, D], BF16)
            nc.sync.dma_start(out=proj_bias_t, in_=proj_bias_d)

        x_ts = [None] * B_LOC
        qk_sbs = [None] * B_LOC
        v_sbs = [None] * B_LOC
        on8s = [None] * B_LOC

        def emit_qk_tile(b, mt):
            st = ps_s.tile([128, N], F32, tag="s")
            for nt in range(QH):
                for kc in range(DC):
                    nc.tensor.matmul(
                        st[:, nt * 512:(nt + 1) * 512],
                        lhsT=w_qk_t[:, kc, mt * 128:(mt + 1) * 128],
                        rhs=x_ts[b][:, kc, nt * 512:(nt + 1) * 512],
                        start=(kc == 0),
                        stop=(kc == DC - 1 and not use_qkv_bias),
                    )
                if use_qkv_bias:
                    nc.tensor.matmul(
                        st[:, nt * 512:(nt + 1) * 512],
                        lhsT=qk_bias_t[:, mt * 128:(mt + 1) * 128],
                        rhs=ones_n[:, nt * 512:(nt + 1) * 512],
                        start=False, stop=True,
                    )
            with nc.allow_low_precision(reason="fp16 activations"):
                nc.vector.tensor_copy(qk_sbs[b][:, mt, :], st)

        def emit_v_tile(b, tt):
            st = ps_s.tile([128, N], F32, tag="s")
            for nt in range(QH):
                for kc in range(DC):
                    nc.tensor.matmul(
                        st[:, nt * 512:(nt + 1) * 512],
                        lhsT=x_ts[b][:, kc, tt * 128:(tt + 1) * 128],
                        rhs=w_v_t[:, kc, nt * 512:(nt + 1) * 512],
                        start=(kc == 0),
                        stop=(kc == DC - 1 and not use_qkv_bias),
                    )
                if use_qkv_bias:
                    nc.tensor.matmul(
                        st[:, nt * 512:(nt + 1) * 512],
                        lhsT=ones_n[:, tt * 128:(tt + 1) * 128],
                        rhs=v_bias_t[:, nt * 512:(nt + 1) * 512],
                        start=False, stop=True,
                    )
            with nc.allow_low_precision(reason="fp16 activations"):
                nc.vector.tensor_copy(v_sbs[b][:, tt, :], st)

        def emit_proj_qt(b, qt):
            st = ps_s.tile([128, N], F32, tag="s")
            for h2 in range(H):
                nc.tensor.matmul(
                    st[:, 0:512],
                    lhsT=on8s[b][:, h2, qt * 128:(qt + 1) * 128],
                    rhs=w_proj_t[:, h2, :],
                    start=(h2 == 0),
                    stop=(h2 == H - 1 and not use_proj_bias),
                )
            if use_proj_bias:
                nc.tensor.matmul(
                    st[:, 0:512],
                    lhsT=ones_row,
                    rhs=proj_bias_t,
                    start=False, stop=True,
                )
            ot = outp.tile([128, 512], F32)
            nc.vector.tensor_copy(ot, st[:, 0:512])
            nc.sync.dma_start(
                out=out_d[b, qt * 128:(qt + 1) * 128, :], in_=ot)

        # ---- load x; minimal qkv prefix for batch 0 ----
        for b in range(B_LOC):
            x_ts[b] = xp.tile([128, DC, N], F16, name="x_t")
            qk_sbs[b] = qkp.tile([128, NT, N], F16, name="qk_sb")
            v_sbs[b] = vp.tile([128, NT, N], BF16, name="v_sb")
            on8s[b] = onp.tile([128, H, N], BF16, name="on8")
        for kc in range(DC):
            nc.sync.dma_start(out=x_ts[0][:, kc, :], in_=xT_d[0, kc])
        emit_qk_tile(0, 0)
        emit_qk_tile(0, 4)
        for kc in range(DC):  # b1's x: off the startup critical path
            nc.sync.dma_start(out=x_ts[1][:, kc, :], in_=xT_d[1, kc])
        nc.sync.dma_start(out=w_proj_t, in_=w_proj_d.transpose([1, 0, 2]))
        for tt in range(NT):
            emit_v_tile(0, tt)

        # ---- attention: head pairs x q-half panels, batch-outer ----
        def emit_epilogue(o0, o1, rs_ps, hp, qh, b):
            h0, h1 = 2 * hp, 2 * hp + 1
            # free the o PSUM banks first
            o_sb = osbp.tile([128, N], F32)
            nc.vector.tensor_copy(o_sb[:, 0:512], o0)
            nc.vector.tensor_copy(o_sb[:, 512:N], o1)
            inv_t = invp.tile([128, 512], F32)
            nc.vector.tensor_copy(inv_t[0:1, :], rs_ps[0:1, :])
            nc.vector.tensor_copy(inv_t[64:65, :], rs_ps[64:65, :])
            nc.sync.dma_start(out=rs_scr[b, hp, qh, 0:512], in_=inv_t[0:1, :])
            nc.sync.dma_start(out=rs_scr[b, hp, qh, 512:N], in_=inv_t[64:65, :])
            # reload as [128, 8]: Newton reciprocal runs partition-parallel
            rsw = invp.tile([128, 8], F32, name="rsw")
            base = ((b * (H // 2) + hp) * QH + qh) * N
            nc.sync.dma_start(out=rsw, in_=bass.AP(
                tensor=rs_scr.tensor, offset=rs_scr.offset + base,
                ap=[[8, 128], [1, 8]]))
            yw = invp.tile([128, 8], F32, name="yw")
            tw = invp.tile([128, 8], F32, name="tw")
            nc.gpsimd.tensor_scalar(
                yw.bitcast(I32), rsw.bitcast(I32),
                scalar1=-1, scalar2=0x7EF311C3,
                op0=mybir.AluOpType.mult, op1=mybir.AluOpType.add)
            for _ in range(2):  # y *= (2 - x*y)
                nc.gpsimd.tensor_tensor(tw, rsw, yw, op=mybir.AluOpType.mult)
                nc.gpsimd.tensor_scalar(
                    tw, tw, scalar1=-1.0, scalar2=2.0,
                    op0=mybir.AluOpType.mult, op1=mybir.AluOpType.add)
                nc.gpsimd.tensor_tensor(yw, yw, tw, op=mybir.AluOpType.mult)
            nc.sync.dma_start(out=inv_scr[b, hp, qh], in_=yw)
            bc_t = bcp.tile([128, N], F32)
            nc.sync.dma_start(out=bc_t, in_=bass.AP(
                tensor=inv_scr.tensor, offset=inv_scr.offset + base,
                ap=[[0, 128], [1, N]]))
            with nc.allow_low_precision(reason="bf16 normalized O^T"):
                nc.gpsimd.tensor_tensor(
                    on8s[b][:, h0, qh * 512:(qh + 1) * 512],
                    o_sb[:, 0:512], bc_t[:, 0:512], op=mybir.AluOpType.mult)
                nc.gpsimd.tensor_tensor(
                    on8s[b][:, h1, qh * 512:(qh + 1) * 512],
                    o_sb[:, 512:N], bc_t[:, 512:N], op=mybir.AluOpType.mult)

        pv_queue = []  # (pt0, pt1, kc, v0, v1, o0, o1, rs_ps, epi)

        def pop_pv():
            pt0, pt1, kcp, v0, v1, o0, o1, rs_ps, epi = pv_queue.pop(0)
            nc.tensor.matmul(o0, lhsT=v0, rhs=pt0,
                             start=(kcp == 0), stop=(kcp == NT - 1))
            nc.tensor.matmul(o1, lhsT=v1, rhs=pt1,
                             start=(kcp == 0), stop=(kcp == NT - 1))
            # rowsum pair col-tiled at PSUM partitions 0/64 -> concurrent
            nc.tensor.matmul(rs_ps[0:1, :], lhsT=ones_col, rhs=pt0,
                             start=(kcp == 0), stop=(kcp == NT - 1),
                             skip_group_check=True)
            nc.tensor.matmul(rs_ps[64:65, :], lhsT=ones_col, rhs=pt1,
                             start=(kcp == 0), stop=(kcp == NT - 1),
                             skip_group_check=True)
            if epi is not None:
                emit_epilogue(*epi)

        for b in range(B_LOC):
            if b == 0:
                fillers = []
                for m2 in range(1, 4):  # head pair m2 needs qk m2 & 4+m2
                    fillers += [(emit_qk_tile, (0, m2)),
                                (emit_qk_tile, (0, 4 + m2))]
                fillers += [(emit_qk_tile, (1, mt)) for mt in range(NT)]
                fillers += [(emit_v_tile, (1, tt)) for tt in range(NT)]
            else:
                fillers = [(emit_proj_qt, (0, qt)) for qt in range(NT)]
            def load_strip(hp2):
                t = stripp.tile([128, 2, STRIP4_W], BF16, name="strip01")
                nc.sync.dma_start(out=t[:, 0, :], in_=strip_d[2 * hp2])
                nc.sync.dma_start(out=t[:, 1, :], in_=strip_d[2 * hp2 + 1])
                return t
            strip_next = load_strip(0)
            for hp in range(H // 2):
                h0, h1 = 2 * hp, 2 * hp + 1
                strip01 = strip_next
                if hp + 1 < H // 2:
                    strip_next = load_strip(hp + 1)
                qT0 = qk_sbs[b][0:64, hp, :]
                kT0 = qk_sbs[b][0:64, 4 + hp, :]
                qT1 = qk_sbs[b][64:128, hp, :]
                kT1 = qk_sbs[b][64:128, 4 + hp, :]
                for qh in range(QH):
                    o0 = ps_o.tile([128, 512], F32, tag="o0")
                    o1 = ps_o.tile([128, 512], F32, tag="o1")
                    rs_ps = ps_rs.tile([65, 512], F32, tag="rs")
                    for kc in range(NT):
                        base_c = (RES - 1 - 4 * kc) * RES + qh * 512
                        st = ps_s.tile([128, N], F32, tag="s")
                        # the two heads' K=64 S matmuls: PE row groups
                        # 0-63 / 64-127 -> execute concurrently
                        nc.tensor.matmul(
                            st[:, 0:512],
                            lhsT=kT0[:, kc * 128:(kc + 1) * 128],
                            rhs=qT0[:, qh * 512:(qh + 1) * 512],
                            start=True, stop=True,
                        )
                        nc.tensor.matmul(
                            st[:, 512:N],
                            lhsT=kT1[:, kc * 128:(kc + 1) * 128],
                            rhs=qT1[:, qh * 512:(qh + 1) * 512],
                            start=True, stop=True,
                        )
                        e_t = ep.tile([128, N], BF16)
                        with nc.allow_low_precision(reason="bf16 exp"):
                            nc.scalar.activation(
                                e_t, st, mybir.ActivationFunctionType.Exp,
                                scale=SCALE)
                        pt_t = ptp.tile([128, N], BF16, name="pt_t")
                        with nc.allow_low_precision(reason="bf16 P^T"):
                            nc.vector.tensor_tensor(
                                pt_t.rearrange("p (a c) -> p a c", a=2),
                                e_t.rearrange("p (a c) -> p a c", a=2),
                                strip01[:, :, base_c:base_c + 512],
                                op=mybir.AluOpType.mult)
                        pt0 = pt_t[:, 0:512]
                        pt1 = pt_t[:, 512:N]
                        epi = ((o0, o1, rs_ps, hp, qh, b)
                               if kc == NT - 1 else None)
                        pv_queue.append(
                            (pt0, pt1, kc,
                             v_sbs[b][:, kc, h0 * 128:(h0 + 1) * 128],
                             v_sbs[b][:, kc, h1 * 128:(h1 + 1) * 128],
                             o0, o1, rs_ps, epi))
                        if len(pv_queue) > 2:
                            pop_pv()
                        if kc in ((2, 5, 7) if b == 0 else (7,)):
                            if fillers:
                                f, args = fillers.pop(0)
                                f(*args)
            for f, args in fillers:
                f(*args)
        while pv_queue:
            pop_pv()

        # ---- projection for batch 1 (batch 0 ran as fillers) ----
        for qt in range(NT):
            emit_proj_qt(1, qt)

    nc.compile()
    return nc


def _prep_core_inputs(x, qkv_w, qkv_b, proj_w, proj_b, attn_biases, bias_idxs):
    """Host-side layout preparation. Returns (shared, per_core_xT, flags)."""
    x = np.ascontiguousarray(np.asarray(x, np.float32))
    qkv_w = np.asarray(qkv_w, np.float32)
    qkv_b = np.asarray(qkv_b, np.float32)
    proj_w = np.asarray(proj_w, np.float32)
    proj_b = np.asarray(proj_b, np.float32)
    attn_biases = np.asarray(attn_biases, np.float32)
    import ml_dtypes
    bf16 = ml_dtypes.bfloat16

    # qkv_w columns: per head 256 = [q 64 | k 64 | v 128]
    Wh = qkv_w.reshape(D, H, 256)
    w_q = Wh[:, :, :DK].reshape(D, H * DK)            # q chans h-major
    w_k = Wh[:, :, DK:2 * DK].reshape(D, H * DK)
    w_qk = np.concatenate([w_q, w_k], axis=1)          # [512, 1024]
    w_v = Wh[:, :, 2 * DK:].reshape(D, H * DV)         # [512, 1024]

    bh = qkv_b.reshape(H, 256)
    qk_bias = np.concatenate([bh[:, :DK].reshape(-1), bh[:, DK:2 * DK].reshape(-1)])
    v_bias = bh[:, 2 * DK:].reshape(-1)

    # strip_h[k1, u*32 + q1] = exp(attn_biases[h])[|u-31|*32 + |k1-q1|]
    E = np.exp(attn_biases)                            # [H, 1024]
    u = np.arange(2 * RES - 1)
    d0 = np.abs(u - (RES - 1))                         # [63]
    r = np.arange(RES)
    rel1 = np.abs(r[:, None] - r[None, :])             # [32, 32] (k1, q1)
    idx = d0[None, :, None] * RES + rel1[:, None, :]   # [32, 63, 32]
    strip0 = E[:, idx.reshape(RES, STRIP_W)]           # [H, 32, 2016]
    # replicated-shifted strip: strip4[h, a*32+k1, w] = strip0[h, k1, w-a*32]
    strip = np.zeros((H, 128, STRIP4_W), np.float32)
    for a in range(4):
        strip[:, a * RES:(a + 1) * RES, a * RES:a * RES + STRIP_W] = strip0
    strip = strip.astype(bf16)

    shared = {
        "ones": np.ones((128, N), bf16),
        "w_qk": np.ascontiguousarray(w_qk.reshape(DC, 128, H * DK * 2)).astype(np.float16),
        "w_v": np.ascontiguousarray(w_v.reshape(DC, 128, H * DV)).astype(np.float16),
        "strip": strip,
        "w_proj": np.ascontiguousarray(proj_w.reshape(H, 128, D)).astype(bf16),
    }
    use_qkv_bias = bool(np.any(qkv_b))
    use_proj_bias = bool(np.any(proj_b))
    if use_qkv_bias:
        shared["qk_bias"] = qk_bias.reshape(1, N).astype(bf16)
        shared["v_bias"] = v_bias.reshape(1, N).astype(bf16)
    if use_proj_bias:
        shared["proj_bias"] = proj_b.reshape(1, D).astype(bf16)

    # x^T per core: [B_LOC, DC, 128, N] fp16
    xT = np.ascontiguousarray(x.transpose(0, 2, 1)).reshape(B, DC, 128, N)
    xT = xT.astype(np.float16)
    per_core = [xT[c * B_LOC:(c + 1) * B_LOC] for c in range(N_CORES)]
    return shared, per_core, use_qkv_bias, use_proj_bias


def kernel(x, qkv_w, qkv_b, proj_w, proj_b, attn_biases, bias_idxs):
    global LAST_RESULT
    shared, per_core, use_qkv_bias, use_proj_bias = _prep_core_inputs(
        x, qkv_w, qkv_b, proj_w, proj_b, attn_biases, bias_idxs)

    nc = build_program(use_qkv_bias, use_proj_bias)

    in_maps = [dict(shared, xT=per_core[c]) for c in range(N_CORES)]
    trace = bool(os.environ.get("BASS_TRACE"))
    res = run_bass_kernel_spmd(nc, in_maps, core_ids=list(range(N_CORES)),
                               trace=trace)
    LAST_RESULT = res
    out = np.concatenate([res.results[c]["out"] for c in range(N_CORES)], axis=0)
    return np.ascontiguousarray(out.astype(np.float32))
